# revision 72
# baseline (speedup 1.0000x reference)
"""Trainium2 Bass kernel for nn_AlibiBlock (dense transformer block with ALiBi).

Contract: kernel(**inputs) takes the FULL unsharded inputs (numpy or jax,
shapes from setup_inputs) and returns the FULL [2, 2048, 1024] float32 output.

Sharding (8 NeuronCores = 2 groups of 4):
  - data parallel over batch (B=2): cores 0-3 <- batch 0, cores 4-7 <- batch 1
  - tensor parallel over heads inside each group for attention (16 heads -> 4
    per core); per query group the proj PARTIALS are computed in NATURAL
    [token, C] orientation (lhsT = y^T tile) and a grouped ReduceScatter
    hands each core the summed update rows for its OWN rank-slice of the
    group, so the residual add is a single DVE op with no transposes.
  - the MLP runs T-parallel: each core processes its T-slice with the FULL
    4096 hidden dim (weights streamed from HBM); the host re-interleaves the
    per-core output slices.  The fc pass is split by token halves so the
    first half's fc work executes UNDER the second ReduceScatter.

Per-core dataflow (T=2048, C=1024, 4 heads of d=64; matmuls bf16 except the
fp8e4m3 DoubleRow paths noted below, fp32 PSUM accumulation, fp32 residual):
  LN1 via bn_stats in natural [T,C] layout (x streamed as bf16);
  PE-transpose -> h^T [C,T] stored fp8;
  qkv^T = Wqkv^T @ h^T as fp8 DoubleRow over k-pairs (weights host-scaled
  x32, descale+bias fused in the DVE eviction);
  attention per head-pair with interleaved kt chains; S^T tiles (bf16)
  restricted to the causally-live columns (diag tiles start at column
  r = kt*128 - qcol); P^T = Exp(S/sqrt(d) - slope*k) per ACT op (analytic
  ALiBi softmax shift, no max pass) written as fp8 into kt-PAIR tiles;
  causal mask = [128,128] lower-triangle multiply on the diagonal block only
  (plus a zeroed wedge in a pair's second subtile); y_aug^T accumulates
  V_aug^T @ P^T via fp8 DoubleRow over kt pairs with a ones-column so row 64
  is the softmax denominator; divide per column half: ACT denominator copy,
  PE ones-broadcast, DVE reciprocal_approx_fast + multiply, interleaved with
  the proj partial chunks;
  proj partials in natural [token, C] orientation -> ReduceScatter
  (token-sliced) -> x2 = (xq + b_proj) + rs_out in one DVE add -> LN2 ->
  h2^T (partly woven into the tail of the last attention group) ->
  merged MLP pipeline per hidden strip: 8 fc matmuls + gelu eviction + 2 fc2
  matmuls (skewed one strip) accumulating the natural-layout output in PSUM;
  token halves pipelined around the second collective with the first strips'
  weights prefetched under attention; final residual add in one DVE op.

LN affine params are folded into the qkv/fc weights on the host; biases are
packed into a single [128, 38] tile (one DMA) plus one replicated-rows tile.
"""

import math
import sys

for _p in ("/opt/trn_rl_repo",):
    if _p not in sys.path:
        sys.path.insert(0, _p)

import numpy as np
import ml_dtypes

import concourse.bass as bass
import concourse.mybir as mybir
import concourse.tile as tile
from concourse import bacc
from concourse.bass_utils import run_bass_kernel_spmd
from concourse.masks import make_identity

BF16 = mybir.dt.bfloat16
F32 = mybir.dt.float32
FP8 = mybir.dt.float8e4
AF = mybir.ActivationFunctionType
WQ_SCALE = 32.0     # host-side fp8 weight scale, undone at qkv eviction

C = 1024            # model dim
NH_LOC = 4          # heads per core
D = 64              # head dim
EPS = 1e-5
NCORES = 8
GROUPS = [[0, 1, 2, 3], [4, 5, 6, 7]]
P = 128
QTW = 512           # matmul free-dim tile (one PSUM bank)
QG = 1024           # query group / pipeline chunk width
TS = QG // 4        # rank slice of a query group



def _build(T: int):
    """Build + compile the SPMD program for sequence length T (multiple of QG)."""
    TPT = T // P        # token partition-tiles
    CT = C // P         # 8
    NQG = T // QG       # query-group chunks
    TQ = T // 4         # T-quarter owned by each core
    QTT = TQ // P       # local token tiles
    GT = TS // P        # local token tiles per query group (2)
    FT = 4 * C // P     # 32 hidden partition-tiles (full MLP hidden)

    nc = bacc.Bacc("TRN2", target_bir_lowering=False, debug=False,
                   num_devices=NCORES)

    x_d = nc.dram_tensor("xbf", [T, C], BF16, kind="ExternalInput")
    xq_d = nc.dram_tensor("xq", [TQ, C], F32, kind="ExternalInput")
    wqkv_d = nc.dram_tensor("wqkv", [C, 3 * NH_LOC * D], FP8, kind="ExternalInput")
    wproj_d = nc.dram_tensor("wproj", [2 * P, C], BF16, kind="ExternalInput")
    wfc_d = nc.dram_tensor("wfc", [C, 4 * C], BF16, kind="ExternalInput")
    wfc2_d = nc.dram_tensor("wfc2", [4 * C, C], BF16, kind="ExternalInput")
    biaspk_d = nc.dram_tensor("biaspk", [P, 38], F32, kind="ExternalInput")
    biasrow_d = nc.dram_tensor("biasrow", [P, 2 * C], F32, kind="ExternalInput")
    alibi_d = nc.dram_tensor("alibi", [P, NH_LOC * TPT], F32, kind="ExternalInput")
    tri_d = nc.dram_tensor("tri", [P, P], BF16, kind="ExternalInput")
    out_d = nc.dram_tensor("out", [TQ, C], F32, kind="ExternalOutput")

    x_t = x_d.ap().rearrange("(n p) c -> n p c", p=P)
    xq_t = xq_d.ap().rearrange("(n p) c -> n p c", p=P)
    out_t = out_d.ap().rearrange("(n p) c -> n p c", p=P)
    # k-PAIR layout for fp8 DoubleRow: tile [P, 2, m] per pair of k-tiles
    wqkv_t = wqkv_d.ap().rearrange("(k j p) m -> k p j m", j=2, p=P)
    wproj_t = wproj_d.ap().rearrange("(k p) m -> k p m", p=P)
    wfc_t = wfc_d.ap().rearrange("(k p) m -> k p m", p=P)
    wfc2_t = wfc2_d.ap().rearrange("(k p) m -> k p m", p=P)

    import contextlib

    with tile.TileContext(nc) as tc, contextlib.ExitStack() as es:
        const = es.enter_context(tc.tile_pool(name="const", bufs=1))
        wpool = es.enter_context(tc.tile_pool(name="wpool", bufs=1))
        dram = es.enter_context(tc.tile_pool(name="dram", bufs=1, space="DRAM"))
        psum = es.enter_context(tc.tile_pool(name="psum", bufs=4, space="PSUM"))
        xs = es.enter_context(tc.tile_pool(name="xs", bufs=4))
        hp = es.enter_context(tc.tile_pool(name="hp", bufs=2))
        bigp = es.enter_context(tc.tile_pool(name="bigp", bufs=1))
        lnp = es.enter_context(tc.tile_pool(name="lnp", bufs=4))
        strip = es.enter_context(tc.tile_pool(name="strip", bufs=2))
        midp = es.enter_context(tc.tile_pool(name="midp", bufs=1))
        x2p = es.enter_context(tc.tile_pool(name="x2p", bufs=2))
        xo = es.enter_context(tc.tile_pool(name="xo", bufs=2))

        # ---- on-chip constants (no DMA) ----
        ident = const.tile([P, P], BF16)
        make_identity(nc, ident)
        eps_sb = const.tile([P, 1], F32)
        nc.vector.memset(eps_sb[:], EPS)
        zrow = const.tile([1, P], BF16)
        nc.vector.memset(zrow[:], 0.0)
        ones64 = const.tile([1, D], BF16)
        nc.vector.memset(ones64[:], 1.0)

        # ---- warmup collective: start the cross-core barrier ASAP ----
        warm_in = dram.tile([4, 1, P], BF16, name="warm_in")
        warm_out = dram.tile([1, P], BF16, name="warm_out")
        for r in range(4):
            nc.gpsimd.dma_start(out=warm_in[r], in_=zrow[:])
        nc.gpsimd.collective_compute(
            "ReduceScatter", mybir.AluOpType.add, replica_groups=GROUPS,
            ins=[warm_in.opt()], outs=[warm_out.opt()])

        hT = bigp.tile([P, CT, T], FP8, name="hT", tag="big")
        qkvT = bigp.tile([P, 6, T], BF16, name="qkvT", tag="qkvT")
        # last dim padded 65 -> 80 so the DoubleRow k-pair stride is 16-aligned
        vaug = bigp.tile([P, NH_LOC, TPT, 80], FP8, name="vaug", tag="vaug")
        yd = [bigp.tile([P, T], BF16, name=f"yd{i}", tag=f"yd{i}")
              for i in range(2)]

        def layernorm_tile(x_tile, h_tile):
            """h = (x - mean)/sqrt(var+eps), bf16 out. x [P, C] fp32."""
            st = lnp.tile([P, 2, 6], F32, name="st")
            xr = x_tile.rearrange("p (a b) -> p a b", a=2)
            for a in range(2):
                nc.vector.bn_stats(out=st[:, a, :], in_=xr[:, a, :])
            mv = lnp.tile([P, 2], F32, name="mv")
            nc.vector.bn_aggr(out=mv[:], in_=st[:])
            rs = lnp.tile([P, 1], F32, name="rs")
            nc.scalar.activation(out=rs[:], in_=mv[:, 1:2], func=AF.Sqrt,
                                 bias=eps_sb[:], scale=1.0)
            nc.vector.reciprocal(out=rs[:], in_=rs[:])
            nc.vector.tensor_scalar(out=h_tile[:], in0=x_tile[:],
                                    scalar1=mv[:, 0:1], scalar2=rs[:],
                                    op0=mybir.AluOpType.subtract,
                                    op1=mybir.AluOpType.mult)

        def transpose_grouped(h_tile, dst, evict):
            """h [P(tok), C] -> dst [P, CT, P] column block (one eviction)."""
            tp = psum.tile([P, CT, P], BF16, tag="sm", name="tp")
            for j in range(CT):
                nc.tensor.transpose(tp[:, j, :], h_tile[:, j * P:(j + 1) * P],
                                    ident[:])
            evict(out=dst, in_=tp[:])

        def ln_tile(tt):
            x_tile = xs.tile([P, C], BF16, name="xsb", tag="xsb")
            nc.sync.dma_start(out=x_tile[:], in_=x_t[tt])
            h_tile = hp.tile([P, C], BF16, name="hp")
            layernorm_tile(x_tile, h_tile)
            transpose_grouped(h_tile, hT[:, :, tt * P:(tt + 1) * P],
                              nc.scalar.copy)

        # ---------- startup: x tiles first, weights on the gpsimd queue ----
        for tt in range(min(4, TPT)):
            ln_tile(tt)

        wqkv_sb = [wpool.tile([P, 2, 3 * NH_LOC * D], FP8, name=f"wqkv{k}")
                   for k in range(CT // 2)]
        for k in range(CT // 2):
            nc.gpsimd.dma_start(out=wqkv_sb[k][:], in_=wqkv_t[k])
        biaspk_sb = const.tile([P, 38], F32)
        nc.gpsimd.dma_start(out=biaspk_sb[:], in_=biaspk_d.ap())
        alibi_sb = const.tile([P, NH_LOC * TPT], F32)
        nc.gpsimd.dma_start(out=alibi_sb[:], in_=alibi_d.ap())
        tri_sb = const.tile([P, P], BF16)
        nc.gpsimd.dma_start(out=tri_sb[:], in_=tri_d.ap())

        # bias rows (b_proj, b_fc2), host-replicated across partitions
        bias2_sb = const.tile([P, 2, C], BF16)
        nc.gpsimd.dma_start(out=bias2_sb[:],
                            in_=biasrow_d.ap().rearrange("p (a c) -> p a c", a=2))
        bproj_bc = bias2_sb[:, 0, :]
        bfc2_bc = bias2_sb[:, 1, :]

        def qkv_half(g, half, m):
            ps = psum.tile([P, QTW], F32, tag="sm", name="ps")
            col = g * QG + half * QTW
            for k in range(CT // 2):
                nc.tensor.matmul(ps[:], wqkv_sb[k][:, :, m * P:(m + 1) * P],
                                 hT[:, 2 * k:2 * k + 2, col:col + QTW],
                                 start=(k == 0), stop=(k == CT // 2 - 1),
                                 perf_mode=mybir.MatmulPerfMode.DoubleRow)
            nc.vector.tensor_scalar(out=qkvT[:, m, col:col + QTW],
                                    in0=ps[:], scalar1=1.0 / WQ_SCALE,
                                    scalar2=biaspk_sb[:, m:m + 1],
                                    op0=mybir.AluOpType.mult,
                                    op1=mybir.AluOpType.add)

        # qkv for the first query group, half 0 (needs hT tiles 0..3 only) —
        # emitted before ln(4..7) so the in-order PE queue reaches it without
        # waiting behind transposes that depend on later x DMAs
        for m in range(6):
            qkv_half(0, 0, m)

        for tt in range(4, min(8, TPT)):
            ln_tile(tt)

        def vaug_chunk(g):
            kts = range(g * 8, min((g + 1) * 8, TPT))
            nk = len(kts)
            for h in range(NH_LOC):
                voff = (h % 2) * D
                tpv = psum.tile([P, nk, D], BF16, tag="sm", name="tpv")
                for i, kt in enumerate(kts):
                    nc.tensor.transpose(
                        tpv[:, i, :],
                        qkvT[voff:voff + D, 4 + h // 2, kt * P:(kt + 1) * P],
                        ident[voff:voff + D, voff:voff + D])
                nc.vector.tensor_copy(out=vaug[:, h, kts.start:kts.stop, 0:D],
                                      in_=tpv[:])
                nc.vector.memset(vaug[:, h, kts.start:kts.stop, D:D + 1], 1.0)

        # qkv half 1 (needs hT tiles 4..7), then vaug; LN tiles 8..15 are
        # deferred into attention(0)'s filler queue so the first S matmuls
        # don't queue behind transposes that wait on late x DMAs
        for m in range(6):
            qkv_half(0, 1, m)
        vaug_chunk(0)

        # per-core wproj rows (my 4 heads), natural layout
        wproj_sb = [wpool.tile([P, C], BF16, name=f"wproj{k}")
                    for k in range(2)]
        for k in range(2):
            nc.gpsimd.dma_start(out=wproj_sb[k][:], in_=wproj_t[k])

        # residual base: xb = xq + b_proj (precomputed off the critical path)
        xb = []
        for tl in range(QTT):
            xq_tile = xs.tile([P, C], F32, name="xq", tag="xq", bufs=2)
            nc.sync.dma_start(out=xq_tile[:], in_=xq_t[tl])
            xbt = midp.tile([P, C], F32, name=f"xb{tl}", tag=f"xb{tl}")
            nc.vector.tensor_add(out=xbt[:], in0=xq_tile[:], in1=bproj_bc[:])
            xb.append(xbt)

        # ReduceScatter buffers per query group (natural token orientation)
        rs_in = [dram.tile([4, GT, P, C], BF16, name=f"rs_in{g}")
                 for g in range(NQG)]
        rs_out = [dram.tile([GT, P, C], BF16, name=f"rs_out{g}")
                  for g in range(NQG)]

        es_w = contextlib.ExitStack()
        wst = es_w.enter_context(tc.tile_pool(name="wst", bufs=12))
        w2st = es_w.enter_context(tc.tile_pool(name="w2st", bufs=5))

        es_attn = contextlib.ExitStack()
        ptp = es_attn.enter_context(tc.tile_pool(name="ptp", bufs=6))
        rbp = es_attn.enter_context(tc.tile_pool(name="rbp", bufs=2))

        def proj_half(g, half):
            """Proj partials for the token tiles of one column half, natural
            [token, C] orientation, evicted bf16 (ACT) into the RS input."""
            for t in range(half * 4, half * 4 + 4):
                col = g * QG + t * P
                ps = psum.tile([P, 2, QTW], F32, tag="acc", bufs=2, name="ps")
                for j in range(2):
                    for hf in range(2):
                        nc.tensor.matmul(
                            ps[:, hf, :], yd[j][:, col:col + P],
                            wproj_sb[j][:, hf * QTW:(hf + 1) * QTW],
                            start=(j == 0), stop=(j == 1))
                st_ = strip.tile([P, C], BF16, name="strip")
                nc.scalar.copy(out=st_[:], in_=ps.rearrange("p a b -> p (a b)"))
                nc.sync.dma_start(out=rs_in[g][t // GT, t % GT], in_=st_[:])

        def attention_headpair(g, hpi, post_half=None, fillq=None,
                               fill_from=0, pops=1):
            """Two heads (2*hpi, 2*hpi+1) with interleaved kt chains so PE can
            run one head's matmuls while ACT runs the other's Exp.  S/PV are
            restricted to causally-live columns (col >= kt*P - qcol).  The
            softmax divide runs per column half; post_half(half) is emitted
            after each half's divides (used to weave in the proj partials).
            fillq is a deque of independent work quanta woven in per kt step
            (from kt >= fill_from) to keep the PE saturated while ACT runs."""
            qcol = g * QG
            KT = 8 * (g + 1)
            NKP = KT // 2            # kt pairs (fp8 DoubleRow PV)
            last_h0p = 4 * g + 2     # number of kt PAIRS feeding half 0
            heads = (2 * hpi, 2 * hpi + 1)
            yps_l, pt_l = {}, {}
            for h in heads:
                yps_l[h] = psum.tile([D + 1, 2, QTW], F32, tag="acc", bufs=2,
                                     name=f"yps{h % 2}")

            def emit_pv(kp, pts):
                c0p = max(2 * kp * P - qcol, 0)
                for h in heads:
                    pt = pts[h]
                    va = vaug[:, h, 2 * kp:2 * kp + 2, 0:D + 1]
                    if c0p < QTW:
                        nc.tensor.matmul(yps_l[h][:, 0, c0p:], va,
                                         pt[:, :, c0p:QTW],
                                         start=(kp == 0),
                                         stop=(kp == last_h0p - 1),
                                         perf_mode=mybir.MatmulPerfMode.DoubleRow)
                        nc.tensor.matmul(yps_l[h][:, 1, :], va,
                                         pt[:, :, QTW:QG],
                                         start=(kp == 0), stop=(kp == NKP - 1),
                                         perf_mode=mybir.MatmulPerfMode.DoubleRow)
                    else:
                        nc.tensor.matmul(yps_l[h][:, 1, c0p - QTW:], va,
                                         pt[:, :, c0p:QG],
                                         start=False, stop=(kp == NKP - 1),
                                         perf_mode=mybir.MatmulPerfMode.DoubleRow)

            pending = None
            for kp in range(NKP):
                for h in heads:
                    pt_l[h] = ptp.tile([P, 2, QG], FP8, name="pt")
                for j in range(2):
                    kt = 2 * kp + j
                    r = kt * P - qcol
                    c0 = max(r, 0)
                    for h in heads:
                        off = (h % 2) * D
                        qT = qkvT[off:off + D, h // 2, :]
                        kT = qkvT[off:off + D, 2 + h // 2, :]
                        bias_ap = alibi_sb[:, h * TPT + kt:h * TPT + kt + 1]
                        pt = pt_l[h]
                        if j == 1 and r > 0:
                            # zero the causally-dead wedge of this subtile so
                            # the paired DoubleRow PV reads zeros there
                            nc.vector.memset(pt[:, 1, max(r - P, 0):c0], 0.0)
                        if c0 < QTW:
                            s0 = psum.tile([P, QTW], F32, tag="sm", name="s0")
                            nc.tensor.matmul(
                                s0[:, c0:], kT[:, kt * P:(kt + 1) * P],
                                qT[:, qcol + c0:qcol + QTW],
                                start=True, stop=True)
                            nc.scalar.activation(
                                out=pt[:, j, c0:QTW], in_=s0[:, c0:],
                                func=AF.Exp, bias=bias_ap,
                                scale=1.0 / math.sqrt(D))
                            s1 = psum.tile([P, QTW], F32, tag="sm", name="s1")
                            nc.tensor.matmul(
                                s1[:], kT[:, kt * P:(kt + 1) * P],
                                qT[:, qcol + QTW:qcol + QG],
                                start=True, stop=True)
                            nc.scalar.activation(
                                out=pt[:, j, QTW:QG], in_=s1[:],
                                func=AF.Exp, bias=bias_ap,
                                scale=1.0 / math.sqrt(D))
                            if r >= 0:
                                v = pt[:, j, r:r + P]
                                nc.vector.tensor_mul(out=v, in0=v,
                                                     in1=tri_sb[:])
                        else:
                            s1 = psum.tile([P, QTW], F32, tag="sm", name="s1")
                            nc.tensor.matmul(
                                s1[:, c0 - QTW:], kT[:, kt * P:(kt + 1) * P],
                                qT[:, qcol + c0:qcol + QG],
                                start=True, stop=True)
                            nc.scalar.activation(
                                out=pt[:, j, c0:QG], in_=s1[:, c0 - QTW:],
                                func=AF.Exp, bias=bias_ap,
                                scale=1.0 / math.sqrt(D))
                            v = pt[:, j, r:r + P]
                            nc.vector.tensor_mul(out=v, in0=v, in1=tri_sb[:])
                if pending is not None:
                    emit_pv(*pending)
                if kp * 2 >= fill_from:
                    for _ in range(pops):
                        if fillq:
                            fillq.popleft()()
                pending = (kp, dict(pt_l))
            emit_pv(*pending)
            # softmax divide: dn row once per head (ACT), then per half a
            # PE broadcast + DVE reciprocal + DVE multiply into yd
            dns = {}
            for h in heads:
                dn = rbp.tile([1, QG], BF16, name="dn")
                nc.scalar.copy(
                    out=dn[:], in_=yps_l[h][D:D + 1, :].rearrange("p a b -> p (a b)"))
                dns[h] = dn
            for hi, h in enumerate(heads):
                off = (h % 2) * D
                for half in range(2):
                    rbps = psum.tile([D, QTW], F32, tag="sm", name="rbps")
                    nc.tensor.matmul(
                        rbps[:], ones64[:],
                        dns[h][:, half * QTW:(half + 1) * QTW],
                        start=True, stop=True)
                    rb = rbp.tile([D, QTW], F32, name="rb")
                    nc.vector.reciprocal_approx_fast(out=rb[:], in_=rbps[:])
                    nc.vector.tensor_mul(
                        out=yd[h // 2][off:off + D,
                                       qcol + half * QTW:qcol + (half + 1) * QTW],
                        in0=yps_l[h][0:D, half, :], in1=rb[:])
                    if hi == 1 and post_half is not None:
                        post_half(half)

        h2Tq = midp.tile([P, CT, TQ], BF16, name="h2Tq", tag="h2Tq")
        # x2b reuses the xb slots (same tag): xb[tl] is dead once x2 is formed
        x2b = [midp.tile([P, C], F32, name=f"x2b{tl}", tag=f"xb{tl}")
               for tl in range(QTT)]

        def post_tile_pre(g, t):
            """PE-free half of the post-RS work for one token tile: readback,
            residual add, LN2, fc2 residual base.  Safe to emit under
            attention (no PE-queue blocking)."""
            tl = g * GT + t
            rt = strip.tile([P, C], BF16, name="strip")
            nc.sync.dma_start(out=rt[:], in_=rs_out[g][t])
            x2 = x2p.tile([P, C], F32, name="x2")
            nc.vector.tensor_add(out=x2[:], in0=xb[tl][:], in1=rt[:])
            h2_tile = hp.tile([P, C], BF16, name="hp")
            layernorm_tile(x2, h2_tile)
            nc.vector.tensor_add(out=x2b[tl][:], in0=x2[:], in1=bfc2_bc[:])
            return h2_tile

        def post_tile_fin(g, t, h2_tile):
            tl = g * GT + t
            transpose_grouped(h2_tile, h2Tq[:, :, tl * P:(tl + 1) * P],
                              nc.vector.tensor_copy)

        def post_group(g, pre=None):
            for t in range(GT):
                h2_tile = pre[t] if pre else post_tile_pre(g, t)
                post_tile_fin(g, t, h2_tile)

        # ---------- MLP machinery (emittable as attention fillers) --------
        fcTq = bigp.tile([P, FT, TQ], BF16, name="fcTq", tag="big")

        def prefetch_mlp_weights():
            """Issue the first wfc/wfc2 loads early (under attention) so the
            first MLP strips don't contend with the ReduceScatter for HBM."""
            wt0 = []
            for k in range(CT):
                w = wst.tile([P, 8 * P], BF16, name="wst")
                nc.gpsimd.dma_start(out=w[:], in_=wfc_t[k][:, 0:8 * P])
                wt0.append(w)
            w20 = []
            for hk in range(5):
                w2 = w2st.tile([P, C], BF16, name="w2st")
                nc.gpsimd.dma_start(out=w2[:], in_=wfc2_t[hk])
                w20.append(w2)
            return {"wt0": wt0, "w20": w20}

        def mlp_pass(g, pre=None):
            """Merged fc -> gelu -> fc2 pipeline for group g's token tiles:
            per hidden strip, 8 fc matmuls + gelu eviction + 2 fc2 matmuls
            accumulating the output in PSUM; wfc/wfc2 stream per strip."""
            pre = pre or {}
            t0, t1 = g * GT * P, (g + 1) * GT * P
            ps2 = [psum.tile([P, 2, QTW], F32, tag="acc", bufs=2,
                             name=f"ps2_{t}") for t in range(GT)]
            def fc2_strip(hk, w2):
                for t in range(GT):
                    tl = g * GT + t
                    for half in range(2):
                        nc.tensor.matmul(
                            ps2[t][:, half, :],
                            fcTq[:, hk, tl * P:(tl + 1) * P],
                            w2[:, half * QTW:(half + 1) * QTW],
                            start=(hk == 0), stop=(hk == FT - 1))

            pending = None     # (hk, w2) one strip behind, hiding gelu latency
            for hg in range(4):
                if hg == 0 and "wt0" in pre:
                    wt = pre["wt0"]
                else:
                    wt = []
                    for k in range(CT):
                        w = wst.tile([P, 8 * P], BF16, name="wst")
                        nc.gpsimd.dma_start(
                            out=w[:],
                            in_=wfc_t[k][:, hg * 8 * P:(hg + 1) * 8 * P])
                        wt.append(w)
                for sp in range(8):
                    hk = hg * 8 + sp
                    ps = psum.tile([P, QTW], F32, tag="sm", name="ps")
                    for k in range(CT):
                        nc.tensor.matmul(ps[:, 0:t1 - t0],
                                         wt[k][:, sp * P:(sp + 1) * P],
                                         h2Tq[:, k, t0:t1],
                                         start=(k == 0), stop=(k == CT - 1))
                    nc.scalar.activation(out=fcTq[:, hk, t0:t1],
                                         in_=ps[:, 0:t1 - t0], func=AF.Gelu,
                                         bias=biaspk_sb[:, 6 + hk:7 + hk],
                                         scale=1.0)
                    if hk < len(pre.get("w20", ())):
                        w2 = pre["w20"][hk]
                    else:
                        w2 = w2st.tile([P, C], BF16, name="w2st")
                        nc.gpsimd.dma_start(out=w2[:], in_=wfc2_t[hk])
                    if pending is not None:
                        fc2_strip(*pending)
                    pending = (hk, w2)
            fc2_strip(*pending)
            for t in range(GT):
                tl = g * GT + t
                o_tile = xo.tile([P, C], F32, name="xo")
                nc.vector.tensor_add(out=o_tile[:], in0=x2b[tl][:],
                                     in1=ps2[t].rearrange("p a b -> p (a b)"))
                nc.sync.dma_start(out=out_t[tl], in_=o_tile[:])

        # ---------- attention + collectives ----------
        from collections import deque
        for g in range(NQG):
            if g + 1 < NQG:
                fillq = deque([lambda tt=tt: ln_tile(tt)
                               for tt in range(8, TPT)] +
                              [lambda gg=g + 1, mm=m: qkv_half(gg, 0, mm)
                               for m in range(6)] +
                              [lambda gg=g + 1, mm=m: qkv_half(gg, 1, mm)
                               for m in range(6)] +
                              [lambda gg=g + 1: vaug_chunk(gg)])
                ff0, ff1, npop = 2, 0, 2
            else:
                fillq = deque()
                ff0 = ff1 = 0
                npop = 1
            attention_headpair(g, 0, fillq=fillq, fill_from=ff0, pops=npop)
            if g > 0:
                # PE-free post-RS(g-1) work overlaps the second headpair
                post_pre = [post_tile_pre(g - 1, t) for t in range(GT)]
            attention_headpair(g, 1, fillq=fillq, fill_from=ff1, pops=npop,
                               post_half=lambda hf, gg=g: proj_half(gg, hf))
            nc.gpsimd.collective_compute(
                "ReduceScatter", mybir.AluOpType.add, replica_groups=GROUPS,
                ins=[rs_in[g].opt()], outs=[rs_out[g].opt()])
            while fillq:
                fillq.popleft()()
            if g == 0 and NQG > 1:
                pre_mlp = prefetch_mlp_weights()
        es_attn.close()

        # post-RS + MLP: group 0's merged fc/fc2 executes under RS(1)
        if NQG > 1:
            post_group(0, pre=post_pre)
            mlp_pass(0, pre_mlp)
            post_group(1)
            mlp_pass(1)
        else:
            post_group(0)
            mlp_pass(0)
        es_w.close()

    nc.compile()
    return nc



def _alibi_slopes(n_head: int) -> np.ndarray:
    def pow2_slopes(n):
        start = 2 ** (-(2 ** (-(math.log2(n) - 3))))
        return [start * start ** i for i in range(n)]
    if math.log2(n_head).is_integer():
        slopes = pow2_slopes(n_head)
    else:
        c = 2 ** math.floor(math.log2(n_head))
        slopes = pow2_slopes(c)
        extra = pow2_slopes(2 * c)[0::2]
        slopes.extend(extra[: n_head - c])
    return np.asarray(slopes, dtype=np.float32)


def make_in_maps(T, x, ln1_w, ln1_b, w_qkv, b_qkv, w_proj, b_proj,
                 ln2_w, ln2_b, w_fc, b_fc, w_fc2, b_fc2, n_head=16):
    bf = ml_dtypes.bfloat16
    f8 = ml_dtypes.float8_e4m3
    TPT = T // P
    slopes = _alibi_slopes(n_head)

    W1 = (ln1_w[:, None] * w_qkv).astype(np.float32)
    b1 = (b_qkv + ln1_b @ w_qkv).astype(np.float32)
    W2 = (ln2_w[:, None] * w_fc).astype(np.float32)
    b2 = (b_fc + ln2_b @ w_fc).astype(np.float32)

    wfc_full = np.ascontiguousarray(W2).astype(bf)        # shared by all cores
    wfc2_full = np.ascontiguousarray(w_fc2).astype(bf)
    biasrow = np.tile(
        np.concatenate([b_proj, b_fc2]).astype(np.float32)[None, :],
        (P, 1))                                            # [P, 2C] replicated
    tri = (np.arange(P)[:, None] <= np.arange(P)[None, :]).astype(bf)
    bfc_pk = np.ascontiguousarray(b2.reshape(32, P).T)     # [P, 32]

    Cq = w_qkv.shape[0]
    in_maps = []
    for c in range(NCORES):
        b, s = c // 4, c % 4
        qs = slice(256 * s, 256 * s + 256)
        wqkv_s = np.concatenate(
            [W1[:, qs], W1[:, Cq + 256 * s: Cq + 256 * s + 256],
             W1[:, 2 * Cq + 256 * s: 2 * Cq + 256 * s + 256]], axis=1)
        bqkv_s = np.concatenate(
            [b1[qs], b1[Cq + 256 * s: Cq + 256 * s + 256],
             b1[2 * Cq + 256 * s: 2 * Cq + 256 * s + 256]])
        biaspk = np.concatenate(
            [np.ascontiguousarray(bqkv_s.reshape(6, P).T), bfc_pk],
            axis=1).astype(np.float32)                     # [P, 38]
        alibi = np.zeros((P, NH_LOC * TPT), np.float32)
        for hl in range(NH_LOC):
            sl = slopes[4 * s + hl]
            for kt in range(TPT):
                alibi[:, hl * TPT + kt] = -sl * (kt * P + np.arange(P))
        in_maps.append({
            "xbf": np.ascontiguousarray(x[b]).astype(bf),
            "xq": np.ascontiguousarray(
                np.concatenate([x[b][g * QG + s * TS: g * QG + (s + 1) * TS]
                                for g in range(T // QG)], axis=0),
                dtype=np.float32),
            "wqkv": (wqkv_s * WQ_SCALE).astype(f8),
            "wproj": np.ascontiguousarray(w_proj[qs, :]).astype(bf),
            "wfc": wfc_full,
            "wfc2": wfc2_full,
            "biaspk": biaspk,
            "biasrow": biasrow,
            "alibi": alibi,
            "tri": tri,
        })
    return in_maps


def assemble(results) -> np.ndarray:
    """Interleave the per-core rank-slices back into [2, T, C]."""
    outs = []
    for b in range(2):
        parts = [np.asarray(results[4 * b + r]["out"]) for r in range(4)]
        TQ, Cc = parts[0].shape
        T = 4 * TQ
        full = np.empty((T, Cc), parts[0].dtype)
        for g in range(T // QG):
            for r in range(4):
                full[g * QG + r * TS: g * QG + (r + 1) * TS] = \
                    parts[r][g * TS:(g + 1) * TS]
        outs.append(full)
    return np.stack(outs)


_nc_cache = {}


def kernel(**inputs) -> np.ndarray:
    inputs = {k: np.asarray(v) for k, v in inputs.items()}
    x = inputs["x"]
    B, T, _ = x.shape
    if T not in _nc_cache:
        _nc_cache[T] = _build(T)
    nc = _nc_cache[T]
    in_maps = make_in_maps(T, **inputs)
    res = run_bass_kernel_spmd(nc, in_maps, core_ids=list(range(NCORES)))
    return assemble(res.results).astype(np.float32)


if __name__ == "__main__":
    rng = np.random.default_rng(0)
    T = 1024
    ins = dict(
        x=rng.standard_normal((2, T, C), dtype=np.float32),
        ln1_w=np.ones(C, np.float32), ln1_b=np.zeros(C, np.float32),
        w_qkv=(rng.standard_normal((C, 3 * C)) * 0.02).astype(np.float32),
        b_qkv=np.zeros(3 * C, np.float32),
        w_proj=(rng.standard_normal((C, C)) * 0.02).astype(np.float32),
        b_proj=np.zeros(C, np.float32),
        ln2_w=np.ones(C, np.float32), ln2_b=np.zeros(C, np.float32),
        w_fc=(rng.standard_normal((C, 4 * C)) * 0.02).astype(np.float32),
        b_fc=np.zeros(4 * C, np.float32),
        w_fc2=(rng.standard_normal((4 * C, C)) * 0.02).astype(np.float32),
        b_fc2=np.zeros(C, np.float32),
    )
    out = kernel(**ins)
    print(out.shape, out.dtype)


# revision 73
# speedup vs baseline: 1.0279x; 1.0279x over previous
"""Trainium2 Bass kernel for nn_AlibiBlock (dense transformer block with ALiBi).

Contract: kernel(**inputs) takes the FULL unsharded inputs (numpy or jax,
shapes from setup_inputs) and returns the FULL [2, 2048, 1024] float32 output.

Sharding (8 NeuronCores = 2 groups of 4):
  - data parallel over batch (B=2): cores 0-3 <- batch 0, cores 4-7 <- batch 1
  - tensor parallel over heads inside each group for attention (16 heads -> 4
    per core); per query group the proj PARTIALS are computed in NATURAL
    [token, C] orientation (lhsT = y^T tile) and a grouped ReduceScatter
    hands each core the summed update rows for its OWN rank-slice of the
    group, so the residual add is a single DVE op with no transposes.
  - the MLP runs T-parallel: each core processes its T-slice with the FULL
    4096 hidden dim (weights streamed from HBM); the host re-interleaves the
    per-core output slices.  The fc pass is split by token halves so the
    first half's fc work executes UNDER the second ReduceScatter.

Per-core dataflow (T=2048, C=1024, 4 heads of d=64; matmuls bf16 except the
fp8e4m3 DoubleRow paths noted below, fp32 PSUM accumulation, fp32 residual):
  LN1 via bn_stats in natural [T,C] layout (x streamed as bf16);
  PE-transpose -> h^T [C,T] stored fp8;
  qkv^T = Wqkv^T @ h^T as fp8 DoubleRow over k-pairs (weights host-scaled
  x32, descale+bias fused in the DVE eviction);
  attention per head-pair with interleaved kt chains; S^T tiles (bf16)
  restricted to the causally-live columns (diag tiles start at column
  r = kt*128 - qcol); P^T = Exp(S/sqrt(d) - slope*k) per ACT op (analytic
  ALiBi softmax shift, no max pass) written as fp8 into kt-PAIR tiles;
  causal mask = [128,128] lower-triangle multiply on the diagonal block only
  (plus a zeroed wedge in a pair's second subtile); y_aug^T accumulates
  V_aug^T @ P^T via fp8 DoubleRow over kt pairs with a ones-column so row 64
  is the softmax denominator; divide per column half: ACT denominator copy,
  PE ones-broadcast, DVE reciprocal_approx_fast + multiply, interleaved with
  the proj partial chunks;
  proj partials in natural [token, C] orientation -> ReduceScatter
  (token-sliced) -> x2 = (xq + b_proj) + rs_out in one DVE add -> LN2 ->
  h2^T (partly woven into the tail of the last attention group) ->
  merged MLP pipeline per hidden strip: 8 fc matmuls + gelu eviction + 2 fc2
  matmuls (skewed one strip) accumulating the natural-layout output in PSUM;
  token halves pipelined around the second collective with the first strips'
  weights prefetched under attention; final residual add in one DVE op.

LN affine params are folded into the qkv/fc weights on the host; biases are
packed into a single [128, 38] tile (one DMA) plus one replicated-rows tile.
"""

import math
import sys

for _p in ("/opt/trn_rl_repo",):
    if _p not in sys.path:
        sys.path.insert(0, _p)

import numpy as np
import ml_dtypes

import concourse.bass as bass
import concourse.mybir as mybir
import concourse.tile as tile
from concourse import bacc
from concourse.bass_utils import run_bass_kernel_spmd
from concourse.masks import make_identity

BF16 = mybir.dt.bfloat16
F32 = mybir.dt.float32
FP8 = mybir.dt.float8e4
AF = mybir.ActivationFunctionType
WQ_SCALE = 32.0     # host-side fp8 weight scale, undone at qkv eviction

C = 1024            # model dim
NH_LOC = 4          # heads per core
D = 64              # head dim
EPS = 1e-5
NCORES = 8
GROUPS = [[0, 1, 2, 3], [4, 5, 6, 7]]
P = 128
QTW = 512           # matmul free-dim tile (one PSUM bank)
QG = 1024           # query group / pipeline chunk width
TS = QG // 4        # rank slice of a query group



def _build(T: int):
    """Build + compile the SPMD program for sequence length T (multiple of QG)."""
    TPT = T // P        # token partition-tiles
    CT = C // P         # 8
    NQG = T // QG       # query-group chunks
    TQ = T // 4         # T-quarter owned by each core
    QTT = TQ // P       # local token tiles
    GT = TS // P        # local token tiles per query group (2)
    FT = 4 * C // P     # 32 hidden partition-tiles (full MLP hidden)

    nc = bacc.Bacc("TRN2", target_bir_lowering=False, debug=False,
                   num_devices=NCORES)

    x_d = nc.dram_tensor("xbf", [T, C], BF16, kind="ExternalInput")
    xq_d = nc.dram_tensor("xq", [TQ, C], F32, kind="ExternalInput")
    wqkv_d = nc.dram_tensor("wqkv", [C, 3 * NH_LOC * D], FP8, kind="ExternalInput")
    wproj_d = nc.dram_tensor("wproj", [2 * P, C], BF16, kind="ExternalInput")
    wfc_d = nc.dram_tensor("wfc", [C, 4 * C], BF16, kind="ExternalInput")
    wfc2_d = nc.dram_tensor("wfc2", [4 * C, C], BF16, kind="ExternalInput")
    biaspk_d = nc.dram_tensor("biaspk", [P, 38], F32, kind="ExternalInput")
    biasrow_d = nc.dram_tensor("biasrow", [P, 2 * C], F32, kind="ExternalInput")
    alibi_d = nc.dram_tensor("alibi", [P, NH_LOC * TPT], F32, kind="ExternalInput")
    tri_d = nc.dram_tensor("tri", [P, P], BF16, kind="ExternalInput")
    out_d = nc.dram_tensor("out", [TQ, C], F32, kind="ExternalOutput")

    x_t = x_d.ap().rearrange("(n p) c -> n p c", p=P)
    xq_t = xq_d.ap().rearrange("(n p) c -> n p c", p=P)
    out_t = out_d.ap().rearrange("(n p) c -> n p c", p=P)
    # k-PAIR layout for fp8 DoubleRow: tile [P, 2, m] per pair of k-tiles
    wqkv_t = wqkv_d.ap().rearrange("(k j p) m -> k p j m", j=2, p=P)
    wproj_t = wproj_d.ap().rearrange("(k p) m -> k p m", p=P)
    wfc_t = wfc_d.ap().rearrange("(k p) m -> k p m", p=P)
    wfc2_t = wfc2_d.ap().rearrange("(k p) m -> k p m", p=P)

    import contextlib

    with tile.TileContext(nc) as tc, contextlib.ExitStack() as es:
        const = es.enter_context(tc.tile_pool(name="const", bufs=1))
        wpool = es.enter_context(tc.tile_pool(name="wpool", bufs=1))
        dram = es.enter_context(tc.tile_pool(name="dram", bufs=1, space="DRAM"))
        psum = es.enter_context(tc.tile_pool(name="psum", bufs=4, space="PSUM"))
        xs = es.enter_context(tc.tile_pool(name="xs", bufs=4))
        hp = es.enter_context(tc.tile_pool(name="hp", bufs=2))
        bigp = es.enter_context(tc.tile_pool(name="bigp", bufs=1))
        lnp = es.enter_context(tc.tile_pool(name="lnp", bufs=4))
        strip = es.enter_context(tc.tile_pool(name="strip", bufs=2))
        midp = es.enter_context(tc.tile_pool(name="midp", bufs=1))
        x2p = es.enter_context(tc.tile_pool(name="x2p", bufs=2))
        xo = es.enter_context(tc.tile_pool(name="xo", bufs=2))

        # ---- on-chip constants (no DMA) ----
        ident = const.tile([P, P], BF16)
        make_identity(nc, ident)
        eps_sb = const.tile([P, 1], F32)
        nc.vector.memset(eps_sb[:], EPS)
        zrow = const.tile([1, P], BF16)
        nc.vector.memset(zrow[:], 0.0)
        ones64 = const.tile([1, D], BF16)
        nc.vector.memset(ones64[:], 1.0)

        # ---- warmup collective: start the cross-core barrier ASAP ----
        warm_in = dram.tile([4, 1, P], BF16, name="warm_in")
        warm_out = dram.tile([1, P], BF16, name="warm_out")
        for r in range(4):
            nc.gpsimd.dma_start(out=warm_in[r], in_=zrow[:])
        nc.gpsimd.collective_compute(
            "ReduceScatter", mybir.AluOpType.add, replica_groups=GROUPS,
            ins=[warm_in.opt()], outs=[warm_out.opt()])

        hT = bigp.tile([P, CT, T], FP8, name="hT", tag="big")
        qkvT = bigp.tile([P, 6, T], BF16, name="qkvT", tag="qkvT")
        # last dim padded 65 -> 80 so the DoubleRow k-pair stride is 16-aligned
        vaug = bigp.tile([P, NH_LOC, TPT, 80], FP8, name="vaug", tag="vaug")
        yd = [bigp.tile([P, T], BF16, name=f"yd{i}", tag=f"yd{i}")
              for i in range(2)]

        def layernorm_tile(x_tile, h_tile):
            """h = (x - mean)/sqrt(var+eps), bf16 out. x [P, C] fp32."""
            st = lnp.tile([P, 2, 6], F32, name="st")
            xr = x_tile.rearrange("p (a b) -> p a b", a=2)
            for a in range(2):
                nc.vector.bn_stats(out=st[:, a, :], in_=xr[:, a, :])
            mv = lnp.tile([P, 2], F32, name="mv")
            nc.vector.bn_aggr(out=mv[:], in_=st[:])
            rs = lnp.tile([P, 1], F32, name="rs")
            nc.scalar.activation(out=rs[:], in_=mv[:, 1:2], func=AF.Sqrt,
                                 bias=eps_sb[:], scale=1.0)
            nc.vector.reciprocal(out=rs[:], in_=rs[:])
            nc.vector.tensor_scalar(out=h_tile[:], in0=x_tile[:],
                                    scalar1=mv[:, 0:1], scalar2=rs[:],
                                    op0=mybir.AluOpType.subtract,
                                    op1=mybir.AluOpType.mult)

        def transpose_grouped(h_tile, dst, evict):
            """h [P(tok), C] -> dst [P, CT, P] column block (one eviction)."""
            tp = psum.tile([P, CT, P], BF16, tag="sm", name="tp")
            for j in range(CT):
                nc.tensor.transpose(tp[:, j, :], h_tile[:, j * P:(j + 1) * P],
                                    ident[:])
            evict(out=dst, in_=tp[:])

        def ln_tile(tt):
            x_tile = xs.tile([P, C], BF16, name="xsb", tag="xsb")
            nc.sync.dma_start(out=x_tile[:], in_=x_t[tt])
            h_tile = hp.tile([P, C], BF16, name="hp")
            layernorm_tile(x_tile, h_tile)
            transpose_grouped(h_tile, hT[:, :, tt * P:(tt + 1) * P],
                              nc.scalar.copy)

        # ---------- startup: x tiles first, weights on the gpsimd queue ----
        for tt in range(min(4, TPT)):
            ln_tile(tt)

        wqkv_sb = [wpool.tile([P, 2, 3 * NH_LOC * D], FP8, name=f"wqkv{k}")
                   for k in range(CT // 2)]
        for k in range(CT // 2):
            nc.gpsimd.dma_start(out=wqkv_sb[k][:], in_=wqkv_t[k])
        biaspk_sb = const.tile([P, 38], F32)
        nc.gpsimd.dma_start(out=biaspk_sb[:], in_=biaspk_d.ap())
        alibi_sb = const.tile([P, NH_LOC * TPT], F32)
        nc.gpsimd.dma_start(out=alibi_sb[:], in_=alibi_d.ap())
        tri_sb = const.tile([P, P], BF16)
        nc.gpsimd.dma_start(out=tri_sb[:], in_=tri_d.ap())

        # bias rows (b_proj, b_fc2), host-replicated across partitions
        bias2_sb = const.tile([P, 2, C], BF16)
        nc.gpsimd.dma_start(out=bias2_sb[:],
                            in_=biasrow_d.ap().rearrange("p (a c) -> p a c", a=2))
        bproj_bc = bias2_sb[:, 0, :]
        bfc2_bc = bias2_sb[:, 1, :]

        def qkv_half(g, half, m):
            ps = psum.tile([P, QTW], F32, tag="sm", name="ps")
            col = g * QG + half * QTW
            for k in range(CT // 2):
                nc.tensor.matmul(ps[:], wqkv_sb[k][:, :, m * P:(m + 1) * P],
                                 hT[:, 2 * k:2 * k + 2, col:col + QTW],
                                 start=(k == 0), stop=(k == CT // 2 - 1),
                                 perf_mode=mybir.MatmulPerfMode.DoubleRow)
            nc.vector.tensor_scalar(out=qkvT[:, m, col:col + QTW],
                                    in0=ps[:], scalar1=1.0 / WQ_SCALE,
                                    scalar2=biaspk_sb[:, m:m + 1],
                                    op0=mybir.AluOpType.mult,
                                    op1=mybir.AluOpType.add)

        # qkv for the first query group, half 0 (needs hT tiles 0..3 only) —
        # emitted before ln(4..7) so the in-order PE queue reaches it without
        # waiting behind transposes that depend on later x DMAs
        for m in range(6):
            qkv_half(0, 0, m)

        for tt in range(4, min(8, TPT)):
            ln_tile(tt)

        def vaug_chunk(g):
            kts = range(g * 8, min((g + 1) * 8, TPT))
            nk = len(kts)
            for h in range(NH_LOC):
                voff = (h % 2) * D
                tpv = psum.tile([P, nk, D], BF16, tag="sm", name="tpv")
                for i, kt in enumerate(kts):
                    nc.tensor.transpose(
                        tpv[:, i, :],
                        qkvT[voff:voff + D, 4 + h // 2, kt * P:(kt + 1) * P],
                        ident[voff:voff + D, voff:voff + D])
                nc.vector.tensor_copy(out=vaug[:, h, kts.start:kts.stop, 0:D],
                                      in_=tpv[:])
                nc.vector.memset(vaug[:, h, kts.start:kts.stop, D:D + 1], 1.0)

        # qkv half 1 (needs hT tiles 4..7), then vaug; LN tiles 8..15 are
        # deferred into attention(0)'s filler queue so the first S matmuls
        # don't queue behind transposes that wait on late x DMAs
        for m in range(6):
            qkv_half(0, 1, m)
        vaug_chunk(0)

        # per-core wproj rows (my 4 heads), natural layout
        wproj_sb = [wpool.tile([P, C], BF16, name=f"wproj{k}")
                    for k in range(2)]
        for k in range(2):
            nc.gpsimd.dma_start(out=wproj_sb[k][:], in_=wproj_t[k])

        # residual base: xb = xq + b_proj (precomputed off the critical path)
        xb = []
        for tl in range(QTT):
            xq_tile = xs.tile([P, C], F32, name="xq", tag="xq", bufs=2)
            nc.sync.dma_start(out=xq_tile[:], in_=xq_t[tl])
            xbt = midp.tile([P, C], F32, name=f"xb{tl}", tag=f"xb{tl}")
            nc.vector.tensor_add(out=xbt[:], in0=xq_tile[:], in1=bproj_bc[:])
            xb.append(xbt)

        # ReduceScatter buffers per query group (natural token orientation)
        rs_in = [dram.tile([4, GT, P, C], BF16, name=f"rs_in{g}")
                 for g in range(NQG)]
        rs_out = [dram.tile([GT, P, C], BF16, name=f"rs_out{g}")
                  for g in range(NQG)]

        es_w = contextlib.ExitStack()
        wst = es_w.enter_context(tc.tile_pool(name="wst", bufs=12))
        w2st = es_w.enter_context(tc.tile_pool(name="w2st", bufs=5))

        es_attn = contextlib.ExitStack()
        ptp = es_attn.enter_context(tc.tile_pool(name="ptp", bufs=6))
        rbp = es_attn.enter_context(tc.tile_pool(name="rbp", bufs=2))

        def proj_half(g, half):
            """Proj partials for the token tiles of one column half, natural
            [token, C] orientation, evicted bf16 (ACT) into the RS input."""
            for t in range(half * 4, half * 4 + 4):
                col = g * QG + t * P
                ps = psum.tile([P, 2, QTW], F32, tag="acc", bufs=2, name="ps")
                for j in range(2):
                    for hf in range(2):
                        nc.tensor.matmul(
                            ps[:, hf, :], yd[j][:, col:col + P],
                            wproj_sb[j][:, hf * QTW:(hf + 1) * QTW],
                            start=(j == 0), stop=(j == 1))
                st_ = strip.tile([P, C], BF16, name="strip")
                nc.scalar.copy(out=st_[:], in_=ps.rearrange("p a b -> p (a b)"))
                nc.sync.dma_start(out=rs_in[g][t // GT, t % GT], in_=st_[:])

        def attention_headpair(g, hpi, post_half=None, fillq=None,
                               fill_from=0, pops=1):
            """Two heads (2*hpi, 2*hpi+1) with interleaved kt chains so PE can
            run one head's matmuls while ACT runs the other's Exp.  S/PV are
            restricted to causally-live columns (col >= kt*P - qcol).  The
            softmax divide runs per column half; post_half(half) is emitted
            after each half's divides (used to weave in the proj partials).
            fillq is a deque of independent work quanta woven in per kt step
            (from kt >= fill_from) to keep the PE saturated while ACT runs."""
            qcol = g * QG
            KT = 8 * (g + 1)
            NKP = KT // 2            # kt pairs (fp8 DoubleRow PV)
            last_h0p = 4 * g + 2     # number of kt PAIRS feeding half 0
            heads = (2 * hpi, 2 * hpi + 1)
            yps_l, pt_l = {}, {}
            for h in heads:
                yps_l[h] = psum.tile([D + 1, 2, QTW], F32, tag="acc", bufs=2,
                                     name=f"yps{h % 2}")

            def emit_pv(kp, pts):
                c0p = max(2 * kp * P - qcol, 0)
                for h in heads:
                    pt = pts[h]
                    va = vaug[:, h, 2 * kp:2 * kp + 2, 0:D + 1]
                    if c0p < QTW:
                        nc.tensor.matmul(yps_l[h][:, 0, c0p:], va,
                                         pt[:, :, c0p:QTW],
                                         start=(kp == 0),
                                         stop=(kp == last_h0p - 1),
                                         perf_mode=mybir.MatmulPerfMode.DoubleRow)
                        nc.tensor.matmul(yps_l[h][:, 1, :], va,
                                         pt[:, :, QTW:QG],
                                         start=(kp == 0), stop=(kp == NKP - 1),
                                         perf_mode=mybir.MatmulPerfMode.DoubleRow)
                    else:
                        nc.tensor.matmul(yps_l[h][:, 1, c0p - QTW:], va,
                                         pt[:, :, c0p:QG],
                                         start=False, stop=(kp == NKP - 1),
                                         perf_mode=mybir.MatmulPerfMode.DoubleRow)

            pending = None
            for kp in range(NKP):
                for h in heads:
                    pt_l[h] = ptp.tile([P, 2, QG], FP8, name="pt")
                for j in range(2):
                    kt = 2 * kp + j
                    r = kt * P - qcol
                    c0 = max(r, 0)
                    for h in heads:
                        off = (h % 2) * D
                        qT = qkvT[off:off + D, h // 2, :]
                        kT = qkvT[off:off + D, 2 + h // 2, :]
                        bias_ap = alibi_sb[:, h * TPT + kt:h * TPT + kt + 1]
                        pt = pt_l[h]
                        if j == 1 and r > 0:
                            # zero the causally-dead wedge of this subtile so
                            # the paired DoubleRow PV reads zeros there
                            nc.vector.memset(pt[:, 1, max(r - P, 0):c0], 0.0)
                        if c0 < QTW:
                            s0 = psum.tile([P, QTW], F32, tag="sm", name="s0")
                            nc.tensor.matmul(
                                s0[:, c0:], kT[:, kt * P:(kt + 1) * P],
                                qT[:, qcol + c0:qcol + QTW],
                                start=True, stop=True)
                            nc.scalar.activation(
                                out=pt[:, j, c0:QTW], in_=s0[:, c0:],
                                func=AF.Exp, bias=bias_ap,
                                scale=1.0 / math.sqrt(D))
                            s1 = psum.tile([P, QTW], F32, tag="sm", name="s1")
                            nc.tensor.matmul(
                                s1[:], kT[:, kt * P:(kt + 1) * P],
                                qT[:, qcol + QTW:qcol + QG],
                                start=True, stop=True)
                            nc.scalar.activation(
                                out=pt[:, j, QTW:QG], in_=s1[:],
                                func=AF.Exp, bias=bias_ap,
                                scale=1.0 / math.sqrt(D))
                            if r >= 0:
                                v = pt[:, j, r:r + P]
                                nc.vector.tensor_mul(out=v, in0=v,
                                                     in1=tri_sb[:])
                        else:
                            s1 = psum.tile([P, QTW], F32, tag="sm", name="s1")
                            nc.tensor.matmul(
                                s1[:, c0 - QTW:], kT[:, kt * P:(kt + 1) * P],
                                qT[:, qcol + c0:qcol + QG],
                                start=True, stop=True)
                            nc.scalar.activation(
                                out=pt[:, j, c0:QG], in_=s1[:, c0 - QTW:],
                                func=AF.Exp, bias=bias_ap,
                                scale=1.0 / math.sqrt(D))
                            v = pt[:, j, r:r + P]
                            nc.vector.tensor_mul(out=v, in0=v, in1=tri_sb[:])
                if pending is not None:
                    emit_pv(*pending)
                if kp * 2 >= fill_from:
                    for _ in range(pops):
                        if fillq:
                            fillq.popleft()()
                pending = (kp, dict(pt_l))
            emit_pv(*pending)
            # softmax divide: dn row once per head (ACT), then per half a
            # PE broadcast + DVE reciprocal + DVE multiply into yd
            dns = {}
            for h in heads:
                dn = rbp.tile([1, QG], BF16, name="dn")
                nc.scalar.copy(
                    out=dn[:], in_=yps_l[h][D:D + 1, :].rearrange("p a b -> p (a b)"))
                dns[h] = dn
            for hi, h in enumerate(heads):
                off = (h % 2) * D
                for half in range(2):
                    rbps = psum.tile([D, QTW], F32, tag="sm", name="rbps")
                    nc.tensor.matmul(
                        rbps[:], ones64[:],
                        dns[h][:, half * QTW:(half + 1) * QTW],
                        start=True, stop=True)
                    rb = rbp.tile([D, QTW], F32, name="rb")
                    nc.vector.reciprocal_approx_fast(out=rb[:], in_=rbps[:])
                    nc.vector.tensor_mul(
                        out=yd[h // 2][off:off + D,
                                       qcol + half * QTW:qcol + (half + 1) * QTW],
                        in0=yps_l[h][0:D, half, :], in1=rb[:])
                    if hi == 1 and post_half is not None:
                        post_half(half)

        h2Tq = midp.tile([P, CT, TQ], BF16, name="h2Tq", tag="h2Tq")
        # x2b reuses the xb slots (same tag): xb[tl] is dead once x2 is formed
        x2b = [midp.tile([P, C], F32, name=f"x2b{tl}", tag=f"xb{tl}")
               for tl in range(QTT)]

        def post_tile(g, t):
            """Local quarter work for one token tile of group g once
            rs_out[g] is ready: residual + LN2 + h2^T + fc2 residual base."""
            tl = g * GT + t
            rt = strip.tile([P, C], BF16, name="strip")
            nc.sync.dma_start(out=rt[:], in_=rs_out[g][t])
            x2 = x2p.tile([P, C], F32, name="x2")
            nc.vector.tensor_add(out=x2[:], in0=xb[tl][:], in1=rt[:])
            h2_tile = hp.tile([P, C], BF16, name="hp")
            layernorm_tile(x2, h2_tile)
            transpose_grouped(h2_tile, h2Tq[:, :, tl * P:(tl + 1) * P],
                              nc.vector.tensor_copy)
            nc.vector.tensor_add(out=x2b[tl][:], in0=x2[:], in1=bfc2_bc[:])

        def post_group(g):
            for t in range(GT):
                post_tile(g, t)

        # ---------- MLP machinery (emittable as attention fillers) --------
        fcTq = bigp.tile([P, FT, TQ], BF16, name="fcTq", tag="big")

        def prefetch_mlp_weights():
            """Issue the first wfc/wfc2 loads early (under attention) so the
            first MLP strips don't contend with the ReduceScatter for HBM."""
            wt0 = []
            for k in range(CT):
                w = wst.tile([P, 8 * P], BF16, name="wst")
                nc.gpsimd.dma_start(out=w[:], in_=wfc_t[k][:, 0:8 * P])
                wt0.append(w)
            w20 = []
            for hk in range(5):
                w2 = w2st.tile([P, C], BF16, name="w2st")
                nc.gpsimd.dma_start(out=w2[:], in_=wfc2_t[hk])
                w20.append(w2)
            return {"wt0": wt0, "w20": w20}

        def mlp_pass(g, pre=None):
            """Merged fc -> gelu -> fc2 pipeline for group g's token tiles:
            per hidden strip, 8 fc matmuls + gelu eviction + 2 fc2 matmuls
            accumulating the output in PSUM; wfc/wfc2 stream per strip."""
            pre = pre or {}
            t0, t1 = g * GT * P, (g + 1) * GT * P
            ps2 = [psum.tile([P, 2, QTW], F32, tag="acc", bufs=2,
                             name=f"ps2_{t}") for t in range(GT)]
            def fc2_strip(hk, w2):
                for t in range(GT):
                    tl = g * GT + t
                    for half in range(2):
                        nc.tensor.matmul(
                            ps2[t][:, half, :],
                            fcTq[:, hk, tl * P:(tl + 1) * P],
                            w2[:, half * QTW:(half + 1) * QTW],
                            start=(hk == 0), stop=(hk == FT - 1))

            pending = None     # (hk, w2) one strip behind, hiding gelu latency
            for hg in range(4):
                if hg == 0 and "wt0" in pre:
                    wt = pre["wt0"]
                else:
                    wt = []
                    for k in range(CT):
                        w = wst.tile([P, 8 * P], BF16, name="wst")
                        nc.gpsimd.dma_start(
                            out=w[:],
                            in_=wfc_t[k][:, hg * 8 * P:(hg + 1) * 8 * P])
                        wt.append(w)
                for sp in range(8):
                    hk = hg * 8 + sp
                    ps = psum.tile([P, QTW], F32, tag="sm", name="ps")
                    for k in range(CT):
                        nc.tensor.matmul(ps[:, 0:t1 - t0],
                                         wt[k][:, sp * P:(sp + 1) * P],
                                         h2Tq[:, k, t0:t1],
                                         start=(k == 0), stop=(k == CT - 1))
                    nc.scalar.activation(out=fcTq[:, hk, t0:t1],
                                         in_=ps[:, 0:t1 - t0], func=AF.Gelu,
                                         bias=biaspk_sb[:, 6 + hk:7 + hk],
                                         scale=1.0)
                    if hk < len(pre.get("w20", ())):
                        w2 = pre["w20"][hk]
                    else:
                        w2 = w2st.tile([P, C], BF16, name="w2st")
                        nc.gpsimd.dma_start(out=w2[:], in_=wfc2_t[hk])
                    if pending is not None:
                        fc2_strip(*pending)
                    pending = (hk, w2)
            fc2_strip(*pending)
            for t in range(GT):
                tl = g * GT + t
                o_tile = xo.tile([P, C], F32, name="xo")
                nc.vector.tensor_add(out=o_tile[:], in0=x2b[tl][:],
                                     in1=ps2[t].rearrange("p a b -> p (a b)"))
                nc.sync.dma_start(out=out_t[tl], in_=o_tile[:])

        # ---------- attention + collectives ----------
        from collections import deque
        for g in range(NQG):
            if g + 1 < NQG:
                fillq = deque([lambda tt=tt: ln_tile(tt)
                               for tt in range(8, TPT)] +
                              [lambda gg=g + 1, mm=m: qkv_half(gg, 0, mm)
                               for m in range(6)] +
                              [lambda gg=g + 1, mm=m: qkv_half(gg, 1, mm)
                               for m in range(6)] +
                              [lambda gg=g + 1: vaug_chunk(gg)])
                ff0, ff1, npop = 2, 0, 2
            else:
                fillq = deque()
                ff0 = ff1 = 0
                npop = 1
            attention_headpair(g, 0, fillq=fillq, fill_from=ff0, pops=npop)
            attention_headpair(g, 1, fillq=fillq, fill_from=ff1, pops=npop,
                               post_half=lambda hf, gg=g: proj_half(gg, hf))
            nc.gpsimd.collective_compute(
                "ReduceScatter", mybir.AluOpType.add, replica_groups=GROUPS,
                ins=[rs_in[g].opt()], outs=[rs_out[g].opt()])
            while fillq:
                fillq.popleft()()
            if g == 0 and NQG > 1:
                pre_mlp = prefetch_mlp_weights()
        es_attn.close()

        # post-RS + MLP: group 0's merged fc/fc2 executes under RS(1)
        if NQG > 1:
            post_group(0)
            mlp_pass(0, pre_mlp)
            post_group(1)
            mlp_pass(1)
        else:
            post_group(0)
            mlp_pass(0)
        es_w.close()

    nc.compile()
    return nc



def _alibi_slopes(n_head: int) -> np.ndarray:
    def pow2_slopes(n):
        start = 2 ** (-(2 ** (-(math.log2(n) - 3))))
        return [start * start ** i for i in range(n)]
    if math.log2(n_head).is_integer():
        slopes = pow2_slopes(n_head)
    else:
        c = 2 ** math.floor(math.log2(n_head))
        slopes = pow2_slopes(c)
        extra = pow2_slopes(2 * c)[0::2]
        slopes.extend(extra[: n_head - c])
    return np.asarray(slopes, dtype=np.float32)


def make_in_maps(T, x, ln1_w, ln1_b, w_qkv, b_qkv, w_proj, b_proj,
                 ln2_w, ln2_b, w_fc, b_fc, w_fc2, b_fc2, n_head=16):
    bf = ml_dtypes.bfloat16
    f8 = ml_dtypes.float8_e4m3
    TPT = T // P
    slopes = _alibi_slopes(n_head)

    W1 = (ln1_w[:, None] * w_qkv).astype(np.float32)
    b1 = (b_qkv + ln1_b @ w_qkv).astype(np.float32)
    W2 = (ln2_w[:, None] * w_fc).astype(np.float32)
    b2 = (b_fc + ln2_b @ w_fc).astype(np.float32)

    wfc_full = np.ascontiguousarray(W2).astype(bf)        # shared by all cores
    wfc2_full = np.ascontiguousarray(w_fc2).astype(bf)
    biasrow = np.tile(
        np.concatenate([b_proj, b_fc2]).astype(np.float32)[None, :],
        (P, 1))                                            # [P, 2C] replicated
    tri = (np.arange(P)[:, None] <= np.arange(P)[None, :]).astype(bf)
    bfc_pk = np.ascontiguousarray(b2.reshape(32, P).T)     # [P, 32]

    Cq = w_qkv.shape[0]
    in_maps = []
    for c in range(NCORES):
        b, s = c // 4, c % 4
        qs = slice(256 * s, 256 * s + 256)
        wqkv_s = np.concatenate(
            [W1[:, qs], W1[:, Cq + 256 * s: Cq + 256 * s + 256],
             W1[:, 2 * Cq + 256 * s: 2 * Cq + 256 * s + 256]], axis=1)
        bqkv_s = np.concatenate(
            [b1[qs], b1[Cq + 256 * s: Cq + 256 * s + 256],
             b1[2 * Cq + 256 * s: 2 * Cq + 256 * s + 256]])
        biaspk = np.concatenate(
            [np.ascontiguousarray(bqkv_s.reshape(6, P).T), bfc_pk],
            axis=1).astype(np.float32)                     # [P, 38]
        alibi = np.zeros((P, NH_LOC * TPT), np.float32)
        for hl in range(NH_LOC):
            sl = slopes[4 * s + hl]
            for kt in range(TPT):
                alibi[:, hl * TPT + kt] = -sl * (kt * P + np.arange(P))
        in_maps.append({
            "xbf": np.ascontiguousarray(x[b]).astype(bf),
            "xq": np.ascontiguousarray(
                np.concatenate([x[b][g * QG + s * TS: g * QG + (s + 1) * TS]
                                for g in range(T // QG)], axis=0),
                dtype=np.float32),
            "wqkv": (wqkv_s * WQ_SCALE).astype(f8),
            "wproj": np.ascontiguousarray(w_proj[qs, :]).astype(bf),
            "wfc": wfc_full,
            "wfc2": wfc2_full,
            "biaspk": biaspk,
            "biasrow": biasrow,
            "alibi": alibi,
            "tri": tri,
        })
    return in_maps


def assemble(results) -> np.ndarray:
    """Interleave the per-core rank-slices back into [2, T, C]."""
    outs = []
    for b in range(2):
        parts = [np.asarray(results[4 * b + r]["out"]) for r in range(4)]
        TQ, Cc = parts[0].shape
        T = 4 * TQ
        full = np.empty((T, Cc), parts[0].dtype)
        for g in range(T // QG):
            for r in range(4):
                full[g * QG + r * TS: g * QG + (r + 1) * TS] = \
                    parts[r][g * TS:(g + 1) * TS]
        outs.append(full)
    return np.stack(outs)


_nc_cache = {}


def kernel(**inputs) -> np.ndarray:
    inputs = {k: np.asarray(v) for k, v in inputs.items()}
    x = inputs["x"]
    B, T, _ = x.shape
    if T not in _nc_cache:
        _nc_cache[T] = _build(T)
    nc = _nc_cache[T]
    in_maps = make_in_maps(T, **inputs)
    res = run_bass_kernel_spmd(nc, in_maps, core_ids=list(range(NCORES)))
    return assemble(res.results).astype(np.float32)


if __name__ == "__main__":
    rng = np.random.default_rng(0)
    T = 1024
    ins = dict(
        x=rng.standard_normal((2, T, C), dtype=np.float32),
        ln1_w=np.ones(C, np.float32), ln1_b=np.zeros(C, np.float32),
        w_qkv=(rng.standard_normal((C, 3 * C)) * 0.02).astype(np.float32),
        b_qkv=np.zeros(3 * C, np.float32),
        w_proj=(rng.standard_normal((C, C)) * 0.02).astype(np.float32),
        b_proj=np.zeros(C, np.float32),
        ln2_w=np.ones(C, np.float32), ln2_b=np.zeros(C, np.float32),
        w_fc=(rng.standard_normal((C, 4 * C)) * 0.02).astype(np.float32),
        b_fc=np.zeros(4 * C, np.float32),
        w_fc2=(rng.standard_normal((4 * C, C)) * 0.02).astype(np.float32),
        b_fc2=np.zeros(C, np.float32),
    )
    out = kernel(**ins)
    print(out.shape, out.dtype)
